# revision 6
# baseline (speedup 1.0000x reference)
"""Betti-matching-loss preprocessing kernel for 8 TRN2 NeuronCores.

Reference computation (per full input of shape (B=4, C=1, D=128, H=256, W=256)):
    pred_super   = 1 - maxpool3d_2x(sigmoid(input))   -> sigmoid is monotone, so
                 = sigmoid(-maxpool3d_2x(input))
    target_super = 1 - (maxpool3d_2x(target) > 0.5)   = (maxpool3d_2x(target) <= 0.5)
    out = stack([pred_super, target_super])           # (2, B, C, 64, 128, 128)

Sharding: pure data parallel. 8 shards = 4 batch samples x 2 D-halves of 64
planes each (the D split at an even index never crosses a pool window).

The kernel is pure HBM-bandwidth bound (measured per-engine SDMA ceiling is
~22 B/ns with 2-KB descriptors, ~355 GB/s/core aggregate), so the win is in
moving fewer bytes:
  * the input is sent as fp16 (max-pooling is exact in any ordered format;
    sigmoid sees |err| <= 2^-11 relative, far inside the 2e-2 gate);
  * binarize-then-pool commutes: maxpool(t) > 0.5  ==  maxpool(t > 0.5), so
    the target is sent as EXACT {0,1} fp16 bits — no precision loss at all;
  * outputs return as fp16 ({0,1} exact for target; sigmoid in fp16 for pred)
    and are upcast on the host.

Per-core layout: chunks of 8 input planes; partition p = 64*s + q holds rows
(4q..4q+3) of planes (d0+4s .. d0+4s+3).  Load lines are 2 KB (the fastest
descriptor geometry measured), 256 descriptors per 512-KB sub-DMA so the
HWDGE ring never backs up into the sequencer.  After the 3-level max tree,
partition p holds output rows (2q, 2q+1) of pooled planes d0/2+2s+{0,1}, so
stores are 512-B contiguous lines via the gpsimd SWDGE queue (which
aggregates them into 4-KB packets).

Queue assignment: input loads on the sync HWDGE ring, target loads on the
scalar HWDGE ring, stores on the independent gpsimd SWDGE queue; the scalar
engine's only other work is the sigmoid, the vector engine does the maxes.
"""

import numpy as np

import bass_rust
import concourse.bass as bass
import concourse.mybir as mybir
import concourse.tile as tile
from concourse.bass_utils import run_bass_kernel_spmd
from concourse.vector_clock import ScopedClock

f16 = mybir.dt.float16


def _patched_drain_and_barrier(self, tick_clock, wait_clock):
    """Replacement for TileContext._drain_and_barrier.

    The stock version hangs every outstanding semaphore wait on one Drain
    instruction; the walrus in this environment rejects >1 sync-wait per
    non-EventSemaphore instruction ("Too many sync wait commands").  Emit
    one sequencer NOP per semaphore wait instead, then drain + barrier.
    """
    ((_, vclock),) = ScopedClock({None: tick_clock.global_clock}).items()
    ticks = list(vclock)
    for proc_idx, sem in self.sems.allocated().items():
        t = ticks[proc_idx]
        if t > 0:
            self.nc.sync.nop()._wait_ge(sem, bass_rust.tick_to_sem(t, proc_idx))
    self.nc.sync.drain()
    self.nc.all_engine_barrier(sem_only=True)
    popped = self.nc._tile_sem_poison_stack.pop()
    assert popped is self._sem_poison
    self.nc.clear_and_free_semaphores(list(self.sems.allocated().values()))


tile.TileContext._drain_and_barrier = _patched_drain_and_barrier


def _split_excess_waits(nc: bass.Bass) -> None:
    """Walrus in this env caps sync-waits at 1 per instruction (2 for
    EventSemaphore).  Move excess waits onto same-engine NoOps inserted
    immediately before the offending instruction."""
    for f in nc.m.functions:
        for bb in f.blocks:
            insts = bb.instructions
            out = []
            changed = False
            for inst in insts:
                si = inst.sync_info
                cap = 2 if type(inst).__name__ == "InstEventSemaphore" else 1
                if si is not None and len(si.on_wait) > cap:
                    w = list(si.on_wait)
                    for k, extra in enumerate(w[cap:]):
                        nop = mybir.InstNoOp(
                            name=f"{inst.name}-xw{k}",
                            engine=inst.engine,
                            sync_info=mybir.SyncInfo(
                                on_wait=[extra], on_update=[]
                            ),
                            bass_nofuse=True,
                        )
                        nc.register_instruction(nop, overwrite=True)
                        out.append(nop)
                    inst.sync_info = mybir.SyncInfo(
                        on_wait=w[:cap], on_update=si.on_update
                    )
                    changed = True
                out.append(inst)
            if changed:
                bb.instructions = out

B, C, D, H, W = 4, 1, 128, 256, 256
NCORES = 8
D_SH = D // 2      # 64 input planes per core
DZ = D_SH // 2     # 32 output planes per core
HO, WO = H // 2, W // 2
PPT = 8            # input planes per chunk (1 MB fp16 loads)


def build_nc(
    d_sh: int = D_SH,
    ppt: int = PPT,
    load_rings=("sync", "scalar"),
    store_ring: str = "gpsimd",
    load_bufs: int = 16,
) -> bass.Bass:
    nt = d_sh // ppt       # chunks per tensor
    dz = d_sh // 2
    nc = bass.Bass()
    inp = nc.declare_dram_parameter("input", [d_sh, H, W], f16, isOutput=False)
    tgt = nc.declare_dram_parameter("target", [d_sh, H, W], f16, isOutput=False)
    out = nc.declare_dram_parameter("out", [2, dz, HO, WO], f16, isOutput=True)

    rings = [getattr(nc, r) for r in load_rings]
    store_eng = getattr(nc, store_ring)

    with tile.TileContext(nc) as tc:
        with (
            tc.tile_pool(name="load", bufs=load_bufs) as load_pool,
            tc.tile_pool(name="lvl1", bufs=3) as pool1,
            tc.tile_pool(name="lvl2", bufs=3) as pool2,
            tc.tile_pool(name="lvl3", bufs=3) as pool3,
            tc.tile_pool(name="post", bufs=8) as pool4,
        ):
            # ---- all loads first: with the whole working set resident
            #      (bufs = 2*nt) no push ever waits on a slot, so both
            #      ring sequencers stream descriptors back-to-back ----
            tiles = {}
            for ci in range(nt):
                d0 = ci * ppt
                for which, src in ((0, inp), (1, tgt)):
                    eng = rings[(2 * ci + which) % len(rings)]
                    # partition 64s+q <- rows 4q..4q+3 of planes
                    # d0+4s..d0+4s+3; 2-KB lines, one sub-DMA per s
                    t = load_pool.tile([128, 4096], f16, tag="load")
                    tiles[ci, which] = t
                    sv = src[d0:d0 + ppt].rearrange(
                        "(s j) (q r) w -> s q j (r w)", j=4, r=4
                    )
                    for s in (0, 1):
                        eng.dma_start(
                            t[s * 64:(s + 1) * 64].rearrange(
                                "p (j lin) -> p j lin", j=4
                            ),
                            sv[s],
                        )

            for ci in range(nt):
                d0 = ci * ppt
                for which in (0, 1):
                    t = tiles[ci, which]

                    # ---- level 1: pool D (plane pairs j=(0,1),(2,3)) ----
                    u = pool1.tile([128, 2048], f16, tag="u")
                    tv = t.rearrange(
                        "p (j2 two lin) -> p j2 two lin", j2=2, two=2
                    )
                    nc.vector.tensor_max(
                        u.rearrange("p (j2 lin) -> p j2 lin", j2=2),
                        tv[:, :, 0, :],
                        tv[:, :, 1, :],
                    )

                    # ---- level 2: pool H (row pairs within partition) ----
                    v = pool2.tile([128, 1024], f16, tag="v")
                    uv = u.rearrange(
                        "p (j2 r2 pr w) -> p j2 r2 pr w", j2=2, r2=2, pr=2
                    )
                    nc.vector.tensor_max(
                        v.rearrange("p (j2 r2 w) -> p j2 r2 w", j2=2, r2=2),
                        uv[:, :, :, 0, :],
                        uv[:, :, :, 1, :],
                    )

                    # ---- level 3: pool W (even/odd columns) ----
                    o = pool3.tile([128, 512], f16, tag="o")
                    vv = v.rearrange(
                        "p (k w2 two) -> p k w2 two", two=2, w2=128
                    )
                    nc.vector.tensor_max(
                        o.rearrange("p (k w2) -> p k w2", w2=128),
                        vv[:, :, :, 0],
                        vv[:, :, :, 1],
                    )

                    # ---- pointwise ----
                    g = pool4.tile([128, 512], f16, tag="g")
                    if which == 0:
                        nc.scalar.activation(
                            g[:], o[:],
                            mybir.ActivationFunctionType.Sigmoid,
                            bias=0.0, scale=-1.0,
                        )
                    else:
                        nc.vector.tensor_scalar(
                            g[:], o[:], 0.5, None, mybir.AluOpType.is_le,
                        )

                    # ---- store: 512-B lines, one sub-DMA per s-group ----
                    for s in (0, 1):
                        z0 = d0 // 2 + 2 * s
                        dst = out[which, z0:z0 + 2].rearrange(
                            "j2 (q r2) w -> q j2 (r2 w)", r2=2
                        )
                        store_eng.dma_start(
                            dst,
                            g[s * 64:(s + 1) * 64].rearrange(
                                "p (j2 lin) -> p j2 lin", j2=2
                            ),
                        )
    _split_excess_waits(nc)
    return nc


_NC_CACHE: dict = {}


def make_in_maps(input: np.ndarray, target: np.ndarray) -> list:
    """Host-side prep: shard batch x D-half, downcast input to fp16, and
    send the target as exact {0,1} fp16 bits (binarize commutes with max)."""
    in_maps = []
    for i in range(NCORES):
        b, half = divmod(i, 2)
        sl = slice(half * D_SH, (half + 1) * D_SH)
        in_maps.append({
            "input": np.ascontiguousarray(input[b, 0, sl], dtype=np.float16),
            "target": (target[b, 0, sl] > 0.5).astype(np.float16),
        })
    return in_maps


def kernel(input: np.ndarray, target: np.ndarray) -> np.ndarray:
    input = np.asarray(input, dtype=np.float32)
    target = np.asarray(target, dtype=np.float32)
    assert input.shape == (B, C, D, H, W), input.shape

    if "nc" not in _NC_CACHE:
        _NC_CACHE["nc"] = build_nc()
    nc = _NC_CACHE["nc"]

    in_maps = make_in_maps(input, target)
    res = run_bass_kernel_spmd(nc, in_maps, core_ids=list(range(NCORES))).results

    full = np.empty((2, B, C, D // 2, HO, WO), dtype=np.float32)
    for i in range(NCORES):
        b, half = divmod(i, 2)
        full[:, b, 0, half * DZ:(half + 1) * DZ] = res[i]["out"]
    return full


# revision 7
# speedup vs baseline: 1.0644x; 1.0644x over previous
"""Betti-matching-loss preprocessing kernel for 8 TRN2 NeuronCores.

Reference computation (per full input of shape (B=4, C=1, D=128, H=256, W=256)):
    pred_super   = 1 - maxpool3d_2x(sigmoid(input))   -> sigmoid is monotone, so
                 = sigmoid(-maxpool3d_2x(input))
    target_super = 1 - (maxpool3d_2x(target) > 0.5)   = (maxpool3d_2x(target) <= 0.5)
    out = stack([pred_super, target_super])           # (2, B, C, 64, 128, 128)

Sharding: pure data parallel. 8 shards = 4 batch samples x 2 D-halves of 64
planes each (the D split at an even index never crosses a pool window).

The kernel is pure HBM-bandwidth bound (measured per-engine SDMA ceiling is
~22 B/ns with 2-KB descriptors, ~355 GB/s/core aggregate), so the win is in
moving fewer bytes:
  * the input is sent as fp16 (max-pooling is exact in any ordered format;
    sigmoid sees |err| <= 2^-11 relative, far inside the 2e-2 gate);
  * binarize-then-pool commutes: maxpool(t) > 0.5  ==  maxpool(t > 0.5), so
    the target is sent as EXACT {0,1} fp16 bits — no precision loss at all;
  * outputs return as fp16 ({0,1} exact for target; sigmoid in fp16 for pred)
    and are upcast on the host.

Per-core layout: chunks of 8 input planes; partition p = 64*s + q holds rows
(4q..4q+3) of planes (d0+4s .. d0+4s+3).  Load lines are 2 KB (the fastest
descriptor geometry measured), 256 descriptors per 512-KB sub-DMA so the
HWDGE ring never backs up into the sequencer.  After the 3-level max tree,
partition p holds output rows (2q, 2q+1) of pooled planes d0/2+2s+{0,1}, so
stores are 512-B contiguous lines via the gpsimd SWDGE queue (which
aggregates them into 4-KB packets).

Queue assignment: input loads on the sync HWDGE ring, target loads on the
scalar HWDGE ring, stores on the independent gpsimd SWDGE queue; the scalar
engine's only other work is the sigmoid, the vector engine does the maxes.
"""

import numpy as np

import bass_rust
import concourse.bass as bass
import concourse.mybir as mybir
import concourse.tile as tile
from concourse.bass_utils import run_bass_kernel_spmd
from concourse.vector_clock import ScopedClock

f16 = mybir.dt.float16


def _patched_drain_and_barrier(self, tick_clock, wait_clock):
    """Replacement for TileContext._drain_and_barrier.

    The stock version hangs every outstanding semaphore wait on one Drain
    instruction; the walrus in this environment rejects >1 sync-wait per
    non-EventSemaphore instruction ("Too many sync wait commands").  Emit
    one sequencer NOP per semaphore wait instead, then drain + barrier.
    """
    ((_, vclock),) = ScopedClock({None: tick_clock.global_clock}).items()
    ticks = list(vclock)
    for proc_idx, sem in self.sems.allocated().items():
        t = ticks[proc_idx]
        if t > 0:
            self.nc.sync.nop()._wait_ge(sem, bass_rust.tick_to_sem(t, proc_idx))
    self.nc.sync.drain()
    self.nc.all_engine_barrier(sem_only=True)
    popped = self.nc._tile_sem_poison_stack.pop()
    assert popped is self._sem_poison
    self.nc.clear_and_free_semaphores(list(self.sems.allocated().values()))


tile.TileContext._drain_and_barrier = _patched_drain_and_barrier


def _split_excess_waits(nc: bass.Bass) -> None:
    """Walrus in this env caps sync-waits at 1 per instruction (2 for
    EventSemaphore).  Move excess waits onto same-engine NoOps inserted
    immediately before the offending instruction."""
    for f in nc.m.functions:
        for bb in f.blocks:
            insts = bb.instructions
            out = []
            changed = False
            for inst in insts:
                si = inst.sync_info
                cap = 2 if type(inst).__name__ == "InstEventSemaphore" else 1
                if si is not None and len(si.on_wait) > cap:
                    w = list(si.on_wait)
                    for k, extra in enumerate(w[cap:]):
                        nop = mybir.InstNoOp(
                            name=f"{inst.name}-xw{k}",
                            engine=inst.engine,
                            sync_info=mybir.SyncInfo(
                                on_wait=[extra], on_update=[]
                            ),
                            bass_nofuse=True,
                        )
                        nc.register_instruction(nop, overwrite=True)
                        out.append(nop)
                    inst.sync_info = mybir.SyncInfo(
                        on_wait=w[:cap], on_update=si.on_update
                    )
                    changed = True
                out.append(inst)
            if changed:
                bb.instructions = out

B, C, D, H, W = 4, 1, 128, 256, 256
NCORES = 8
D_SH = D // 2      # 64 input planes per core
DZ = D_SH // 2     # 32 output planes per core
HO, WO = H // 2, W // 2
PPT = 8            # input planes per chunk (1 MB fp16 loads)


def build_nc(
    d_sh: int = D_SH,
    ppt: int = PPT,
    load_rings=("sync",),
    store_ring: str = "gpsimd",
    load_bufs: int = 16,
) -> bass.Bass:
    nt = d_sh // ppt       # chunks per tensor
    dz = d_sh // 2
    nc = bass.Bass()
    inp = nc.declare_dram_parameter("input", [d_sh, H, W], f16, isOutput=False)
    tgt = nc.declare_dram_parameter("target", [d_sh, H, W], f16, isOutput=False)
    out = nc.declare_dram_parameter("out", [2, dz, HO, WO], f16, isOutput=True)

    rings = [getattr(nc, r) for r in load_rings]
    store_eng = getattr(nc, store_ring)

    with tile.TileContext(nc) as tc:
        with (
            tc.tile_pool(name="load", bufs=load_bufs) as load_pool,
            tc.tile_pool(name="lvl1", bufs=3) as pool1,
            tc.tile_pool(name="lvl2", bufs=3) as pool2,
            tc.tile_pool(name="lvl3", bufs=3) as pool3,
            tc.tile_pool(name="post", bufs=8) as pool4,
        ):
            # ---- all loads first: with the whole working set resident
            #      (bufs = 2*nt) no push ever waits on a slot, so both
            #      ring sequencers stream descriptors back-to-back ----
            tiles = {}
            for ci in range(nt):
                d0 = ci * ppt
                for which, src in ((0, inp), (1, tgt)):
                    eng = rings[(2 * ci + which) % len(rings)]
                    # partition 64s+q <- rows 4q..4q+3 of planes
                    # d0+4s..d0+4s+3; 2-KB lines, one sub-DMA per s
                    t = load_pool.tile([128, 4096], f16, tag="load")
                    tiles[ci, which] = t
                    sv = src[d0:d0 + ppt].rearrange(
                        "(s j) (q r) w -> s q j (r w)", j=4, r=4
                    )
                    for s in (0, 1):
                        eng.dma_start(
                            t[s * 64:(s + 1) * 64].rearrange(
                                "p (j lin) -> p j lin", j=4
                            ),
                            sv[s],
                        )

            for ci in range(nt):
                d0 = ci * ppt
                for which in (0, 1):
                    t = tiles[ci, which]

                    # ---- level 1: pool D (plane pairs j=(0,1),(2,3)) ----
                    u = pool1.tile([128, 2048], f16, tag="u")
                    tv = t.rearrange(
                        "p (j2 two lin) -> p j2 two lin", j2=2, two=2
                    )
                    nc.vector.tensor_max(
                        u.rearrange("p (j2 lin) -> p j2 lin", j2=2),
                        tv[:, :, 0, :],
                        tv[:, :, 1, :],
                    )

                    # ---- level 2: pool H (row pairs within partition) ----
                    v = pool2.tile([128, 1024], f16, tag="v")
                    uv = u.rearrange(
                        "p (j2 r2 pr w) -> p j2 r2 pr w", j2=2, r2=2, pr=2
                    )
                    nc.vector.tensor_max(
                        v.rearrange("p (j2 r2 w) -> p j2 r2 w", j2=2, r2=2),
                        uv[:, :, :, 0, :],
                        uv[:, :, :, 1, :],
                    )

                    # ---- level 3: pool W (even/odd columns) ----
                    o = pool3.tile([128, 512], f16, tag="o")
                    vv = v.rearrange(
                        "p (k w2 two) -> p k w2 two", two=2, w2=128
                    )
                    nc.vector.tensor_max(
                        o.rearrange("p (k w2) -> p k w2", w2=128),
                        vv[:, :, :, 0],
                        vv[:, :, :, 1],
                    )

                    # ---- pointwise ----
                    g = pool4.tile([128, 512], f16, tag="g")
                    if which == 0:
                        nc.scalar.activation(
                            g[:], o[:],
                            mybir.ActivationFunctionType.Sigmoid,
                            bias=0.0, scale=-1.0,
                        )
                    else:
                        nc.vector.tensor_scalar(
                            g[:], o[:], 0.5, None, mybir.AluOpType.is_le,
                        )

                    # ---- store: 512-B lines, one sub-DMA per s-group ----
                    for s in (0, 1):
                        z0 = d0 // 2 + 2 * s
                        dst = out[which, z0:z0 + 2].rearrange(
                            "j2 (q r2) w -> q j2 (r2 w)", r2=2
                        )
                        store_eng.dma_start(
                            dst,
                            g[s * 64:(s + 1) * 64].rearrange(
                                "p (j2 lin) -> p j2 lin", j2=2
                            ),
                        )
    _split_excess_waits(nc)
    return nc


_NC_CACHE: dict = {}


def make_in_maps(input: np.ndarray, target: np.ndarray) -> list:
    """Host-side prep: shard batch x D-half, downcast input to fp16, and
    send the target as exact {0,1} fp16 bits (binarize commutes with max)."""
    in_maps = []
    for i in range(NCORES):
        b, half = divmod(i, 2)
        sl = slice(half * D_SH, (half + 1) * D_SH)
        in_maps.append({
            "input": np.ascontiguousarray(input[b, 0, sl], dtype=np.float16),
            "target": (target[b, 0, sl] > 0.5).astype(np.float16),
        })
    return in_maps


def kernel(input: np.ndarray, target: np.ndarray) -> np.ndarray:
    input = np.asarray(input, dtype=np.float32)
    target = np.asarray(target, dtype=np.float32)
    assert input.shape == (B, C, D, H, W), input.shape

    if "nc" not in _NC_CACHE:
        _NC_CACHE["nc"] = build_nc()
    nc = _NC_CACHE["nc"]

    in_maps = make_in_maps(input, target)
    res = run_bass_kernel_spmd(nc, in_maps, core_ids=list(range(NCORES))).results

    full = np.empty((2, B, C, D // 2, HO, WO), dtype=np.float32)
    for i in range(NCORES):
        b, half = divmod(i, 2)
        full[:, b, 0, half * DZ:(half + 1) * DZ] = res[i]["out"]
    return full
